# revision 38
# baseline (speedup 1.0000x reference)
"""Bass/Trainium2 kernel for nn_ADJ_FirstLayer (gnn_message_passing).

reference(x):  N = x.shape[0]; M = N + 4
  A = eye(M); A[N:, N:] = 1  (symmetric)
  d = rowsum(A)^-0.5  ->  d[i] = 1 for i < N, 0.5 for i >= N
  out = d[:,None] * A.T * d[None,:]
  => out = identity on first N diagonal entries, bottom-right 4x4 block = 0.25

The output depends only on N, not on x's values, and is 99.99% zeros:
a pure HBM-write-bandwidth problem (M*M*4 bytes = 268.7 MB).

Sharding: row-shard the (M x M) output across 8 cores, R = 1025 rows each
(8*1025 = 8200 >= 8196; the host trims the last 4 garbage rows). Each core
zero-fills its (R, M) block with large SBUF->DRAM DMAs, then writes its
piece of the diagonal with a dynamic-offset strided DMA (stride M+1). All
rank-dependence (diagonal column offset, diagonal values, 4x4 corner
values) is carried in tiny per-core input arrays; the SPMD program is
identical on every core. x itself never touches the device - it does not
appear in the math.

Measured-on-HW design notes (fast-mode HW exec ~96 us; contention mode
~110 us; do-nothing-NEFF floor ~12.7 us):
 - The NEFF preamble (EVSEM butterfly + tensor loads) keeps every engine
   busy until ~7.6 us; nothing user-issued can start earlier. Sync (SP)
   issues first; Vector/GpSimd wake ~7.6-8.9 us.
 - The zero body is streamed as A + B. A (512 KB) is DRAM->DRAM from a
   host-zero input: no SBUF dependency, so it streams from ~8.3 us while
   the zero-tile memsets (split asymmetrically across GpSimd and Vector)
   finish; B (33 MB) is ONE broadcast-source DMA (the [128, 4032] zeroed
   SBUF span re-read 16x via a stride-0 middle dim, ~15.75 KB
   descriptors).
 - Descriptor size matters (2 KB descs cost ~12%, ~11 ns/descriptor),
   but the residual ~5 us wall-vs-busy gap inside the stream is
   invariant to descriptor size AND to dst traversal order (bank-spread
   tested) - it is HBM-side pacing, not addressable from the kernel.
 - Splitting the stream across two HWDGE rings (sync+scalar) REGRESSED
   ~17% (engine round-robin stalls); only tiny DMAs (remainder, tail,
   corner block) go on the scalar ring, in parallel with the sync ring's
   main-diagonal write.
 - The 1024-element main diagonal segment reads its values from a
   [128, 8] SBUF tile so the HWDGE spreads the 1024 single-element
   descriptors across all 16 SDMA engines (a [1, 1024] source pins them
   all to engine 0: ~11 us serial tail).
 - Row 1024's diagonal element is written separately (1-element DMA at a
   second dynamic offset); on core 7 that write is aimed at a trimmed
   garbage row.
"""
import sys

if "/opt/trn_rl_repo" not in sys.path:
    sys.path.insert(0, "/opt/trn_rl_repo")

import numpy as np

import concourse.bass as bass
from concourse import mybir
from concourse.bass_utils import run_bass_kernel_spmd


def _ensure_axon_hooks():
    """bass_utils' trace path does `from antenv.axon_hooks import ...`
    unconditionally; this image's antenv lacks that module, which would
    crash any BASS_TRACE=1 run. Inject it (with the ctypes NTFF hook when
    available) so tracing works instead of raising."""
    import types

    if "antenv.axon_hooks" in sys.modules:
        return
    hook = None
    try:
        if "/root/.axon_site" not in sys.path:
            sys.path.insert(0, "/root/.axon_site")
        from trn_agent_boot.trn_boot import _ntff_profile_via_ctypes

        hook = _ntff_profile_via_ctypes("/opt/axon/libaxon_pjrt.so")
    except Exception:
        hook = None
    mod = types.ModuleType("antenv.axon_hooks")
    mod._hook = hook
    mod.get_axon_ntff_profile_hook = lambda: mod._hook
    mod.set_axon_ntff_profile_hook = lambda h: setattr(mod, "_hook", h)
    sys.modules["antenv.axon_hooks"] = mod


_ensure_axon_hooks()

N = 8192
M = N + 4            # 8196
N_CORES = 8
R = 1025             # rows per core; 8*1025 = 8200, host trims to 8196
FLAT = R * M         # 8,400,900 elements per core

DIAG_MAIN = 1024     # main diagonal segment length = 128 partitions * 8
BLK_ROW0 = 1017      # local row of the 4x4 ones block on core 7

ZT = 4096            # zero-tile SBUF extent [128, ZT] f32
BIG = 128 * 16 * ZT               # 8,388,608 elements (32 MB) zero body
REM = FLAT - BIG                  # 12,292 = 4 * 3073
REM_P, REM_F = 4, 3073
# The zero body is written as A + B:
#  A: 512 KB DRAM->DRAM from a host-zero input - needs no SBUF, so it
#     launches at engine wake (~8 us) and streams while the memsets run.
#  B: the rest, broadcast-sourced from the SBUF zero tile (span 4032,
#     15.75 KB descriptors), issued as soon as the memsets finish.
A_ELEMS = 128 * 1024              # 131,072 elements (512 KB)
B_SPAN = 4032                     # SBUF source span for B
B_REP = 16                        # (BIG - A_ELEMS) = 128 * B_REP * B_SPAN
MS_G = 2304          # gpsimd memsets [0:MS_G), vector [MS_G:B_SPAN)
                     # (gpsimd wakes ~0.9us earlier; finish ~10.1/10.4 us)

C1_MAX = (N_CORES - 2) * R + DIAG_MAIN * (M + 1)  # largest tail offset (core 6)

_nc_cache = None


def _build():
    nc = bass.Bass()
    zsrc = nc.declare_dram_parameter("zsrc", [128, 1024], mybir.dt.float32, isOutput=False)
    dvals = nc.declare_dram_parameter("dvals", [128, 8], mybir.dt.float32, isOutput=False)
    svals = nc.declare_dram_parameter("svals", [1, 24], mybir.dt.float32, isOutput=False)
    offs = nc.declare_dram_parameter("offs", [1, 2], mybir.dt.int32, isOutput=False)
    out = nc.declare_dram_parameter("out", [R, M], mybir.dt.float32, isOutput=True)
    out_flat = out[:].flatten()

    with (
        nc.Block() as block,
        nc.semaphore("prep_sem") as prep_sem,
        nc.semaphore("in_sem") as in_sem,
        nc.semaphore("zdma_sem") as zdma_sem,
        nc.semaphore("fdma_sem") as fdma_sem,
        nc.sbuf_tensor("ztile", [128, ZT], mybir.dt.float32) as ztile,
        nc.sbuf_tensor("dtile", [128, 8], mybir.dt.float32) as dtile,
        nc.sbuf_tensor("stile", [1, 24], mybir.dt.float32) as stile,
        nc.sbuf_tensor("otile", [1, 2], mybir.dt.int32) as otile,
        nc.sync.register() as r0,
        nc.scalar.register() as r1,
    ):
        # asymmetric split: gpsimd wakes ~0.9us before vector (measured),
        # so give it a head-start-sized share despite its slower rate
        @block.vector
        def _(vector):
            vector.memset(ztile[:, MS_G:B_SPAN], 0.0).then_inc(prep_sem, 1)

        @block.gpsimd
        def _(gpsimd):
            gpsimd.memset(ztile[:, 0:MS_G], 0.0).then_inc(prep_sem, 1)

        @block.scalar
        def _(scalar):
            # remainder (12,292 elements) on the ACT HWDGE ring so it does
            # not delay engine 0's share of the big stream
            scalar.wait_ge(prep_sem, 2)
            dst = bass.AP(out_flat.tensor, BIG, [[REM_F, REM_P], [1, REM_F]])
            scalar.dma_start(out=dst, in_=ztile[0:REM_P, 0:REM_F]).then_inc(zdma_sem, 16)
            # row-1024 diagonal element + 4x4 corner block, in parallel with
            # the sync ring's main-diagonal write
            scalar.wait_ge(in_sem, 48)
            scalar.reg_load(r1, otile[0:1, 1:2])
            c1 = scalar.snap(r1)
            d1 = out_flat[0 : C1_MAX + 1][bass.ds(c1, 1)].offset
            tail_ap = bass.AP(out_flat.tensor, d1, [[1, 1]])
            scalar.wait_ge(zdma_sem, 48)
            scalar.dma_start(out=tail_ap, in_=stile[0:1, 0:1]).then_inc(fdma_sem, 16)
            scalar.dma_start(
                out=out[BLK_ROW0 : BLK_ROW0 + 4, N : N + 4],
                in_=stile[0:1, 4:20],
            ).then_inc(fdma_sem, 16)

        @block.sync
        def _(sync):
            # A: DRAM->DRAM zeros, no SBUF dependency - streams immediately
            dst = bass.AP(out_flat.tensor, 0, [[1, A_ELEMS]])
            sync.dma_start(out=dst, in_=zsrc[:, :]).then_inc(zdma_sem, 16)
            sync.dma_start(out=dtile[:, :], in_=dvals[:, :]).then_inc(in_sem, 16)
            sync.dma_start(out=stile[:, :], in_=svals[:, :]).then_inc(in_sem, 16)
            sync.dma_start(out=otile[:, :], in_=offs[:, :]).then_inc(in_sem, 16)
            sync.wait_ge(prep_sem, 2)
            zsem = 32  # A + scalar's remainder DMA
            zap = ztile[:, :]
            # B: the remaining 31.5 MB, broadcast-sourced; bank-spread
            # traversal (consecutive descriptors of one engine jump ~2 MB)
            dst = bass.AP(out_flat.tensor, A_ELEMS,
                          [[B_SPAN, 128], [128 * B_SPAN, B_REP], [1, B_SPAN]])
            src = bass.AP(zap.tensor, zap.offset,
                          [[zap.ap[0][0], 128], [0, B_REP], [1, B_SPAN]])
            sync.dma_start(out=dst, in_=src).then_inc(zdma_sem, 16)
            zsem += 16

            # load diagonal offset while the zero stream runs
            sync.wait_ge(in_sem, 48)
            sync.reg_load(r0, otile[0:1, 0:1])
            c0 = sync.snap(r0)
            d0 = out_flat[0 : (N_CORES - 1) * R + 1][bass.ds(c0, 1)].offset
            main_ap = bass.AP(out_flat.tensor, d0, [[M + 1, DIAG_MAIN]])

            sync.wait_ge(zdma_sem, zsem)
            # main diagonal at dynamic offset (stride M+1 walks the diagonal)
            with nc.allow_non_contiguous_dma(reason="diagonal scatter"):
                sync.dma_start(out=main_ap, in_=dtile[:, :]).then_inc(fdma_sem, 16)
            sync.wait_ge(fdma_sem, 48)
    return nc


def _in_maps():
    maps = []
    zsrc = np.zeros((128, 1024), np.float32)
    for r in range(N_CORES):
        dvals = np.ones((128, 8), np.float32)
        svals = np.zeros((1, 24), np.float32)
        offs = np.zeros((1, 2), np.int32)
        c0 = r * R
        if r < N_CORES - 1:
            svals[0, 0] = 1.0                  # row-1024 diagonal element
            c1 = c0 + DIAG_MAIN * (M + 1)
        else:
            # core 7: global rows 7175..8199; 8192..8195 hold the ones-block,
            # 8196..8199 are trimmed garbage.
            flat = dvals.reshape(-1)
            flat[BLK_ROW0 : BLK_ROW0 + 4] = 0.25   # diag entries in the 4x4 block
            flat[BLK_ROW0 + 4 :] = 0.0             # rows 8196+: garbage, any value
            svals[0, 0] = 0.0
            svals[0, 4:20] = 0.25              # the 4x4 ones block * 0.25
            c1 = (DIAG_MAIN - 3) * M           # inside garbage row 1021
        offs[0, 0] = c0
        offs[0, 1] = c1
        maps.append({"zsrc": zsrc, "dvals": dvals, "svals": svals, "offs": offs})
    return maps


def _run(trace=False, **kwargs):
    global _nc_cache
    if _nc_cache is None:
        _nc_cache = _build()
    return run_bass_kernel_spmd(
        _nc_cache, _in_maps(), core_ids=list(range(N_CORES)), trace=trace, **kwargs
    )


def kernel(x: np.ndarray) -> np.ndarray:
    assert x.shape == (N, 2048), x.shape
    res = _run()
    blocks = [res.results[r]["out"] for r in range(N_CORES)]
    return np.concatenate(blocks, axis=0)[:M]


if __name__ == "__main__":
    out = kernel(np.zeros((N, 2048), np.float32))
    print(out.shape, out.dtype)
